# revision 11
# baseline (speedup 1.0000x reference)
"""Trainium2 Bass kernel for the Anderson-accelerated DEQ block.

Math (per reference):
    xp = x @ Wx + b
    z_0 = 0
    for i in 0..5:
        fz = tanh(z_i @ Wz + xp)          # i==0: tanh(xp)
        g_i = fz - z_i
        u_i = z_i + 0.9 g_i
        if i < 2:  z_{i+1} = u_i
        else:
            s_gg  = rowsum(g_i * g_i)
            s_ggp = rowsum(g_i * g_{i-1})
            num   = s_gg - s_ggp                      # == rowsum(DG*g)
            den   = s_gg - 2 s_ggp + s_gg_prev + LAM  # == rowsum(DG*DG)+LAM
            gamma = num / den
            z_{i+1} = u_i - gamma * (u_i - u_{i-1})   # == z+.9g-gamma(DZ+.9DG)
    return z_6

Sharding: data-parallel over batch. 8 cores x 128 rows each; Wz/Wx/b
replicated per core. No cross-core communication.

Layout per core: row-tensors are [128 partitions = rows, 2048 free = D].
GEMM uses zT (transposed via PE) as the stationary operand and Wz rows as
the moving operand, accumulating n-chunks of 512 into 4 PSUM banks.
All matmuls run as float32r (full fp32 precision via PE replication,
1 cycle/row at moving-dim 512).
"""

import numpy as np

import concourse.bass as bass
import concourse.bacc as bacc
import concourse.mybir as mybir
import concourse.tile as tile
from concourse.masks import make_identity

AF = mybir.ActivationFunctionType
OP = mybir.AluOpType
F32 = mybir.dt.float32
F32R = mybir.dt.float32r

N_CORES = 8
B, D = 1024, 2048
BS = B // N_CORES       # 128 rows per core
P = 128
NK = D // P             # 16 contraction chunks
NCH = 4                 # column chunks
CW = D // NCH           # 512
NK_RES = 14             # Wz chunks kept resident in SBUF
BETA = 0.9
LAM = 1e-4
MAX_ITER = 6


def _r(ap):
    return ap  # tiles already float32r


def _emit(tc, ctx, x_d, wz_d, wx_d, b_d, out_d):
    nc = tc.nc

    const = ctx.enter_context(tc.tile_pool(name="const", bufs=1))
    wzp = ctx.enter_context(tc.tile_pool(name="wzp", bufs=NK_RES))
    wxp = ctx.enter_context(tc.tile_pool(name="wxp", bufs=2))
    state = ctx.enter_context(tc.tile_pool(name="state", bufs=2))
    io = ctx.enter_context(tc.tile_pool(name="io", bufs=1))
    chk = ctx.enter_context(tc.tile_pool(name="chk", bufs=2))
    ztp = ctx.enter_context(tc.tile_pool(name="ztp", bufs=4))
    smp = ctx.enter_context(tc.tile_pool(name="smp", bufs=2))
    mmp = ctx.enter_context(tc.tile_pool(name="mmp", bufs=4, space="PSUM"))
    tpp = ctx.enter_context(tc.tile_pool(name="tpp", bufs=2, space="PSUM"))

    # constants
    ident = const.tile([P, P], F32, name="ident")
    make_identity(nc, ident)
    zbias = const.tile([P, 1], F32, name="zbias")
    nc.gpsimd.memset(zbias[:], 0.0)
    ones_f32 = const.tile([1, P], F32, name="ones_f32")
    nc.gpsimd.memset(ones_f32[:], 1.0)
    ones_row = const.tile([1, P], F32R, name="ones_row")
    nc.scalar.copy(ones_row[:], ones_f32[:])
    # b shares an io slot with xp (b is consumed by the rank-1 bias matmuls
    # before xp's first write, so one slot suffices)
    b_sb = io.tile([1, D], F32R, name="b_sb", tag="bxp")
    nc.sync.dma_start(b_sb[:], b_d[:])

    # x rides a Wx-stream slot: it is dead after the x transposes
    x_sb = wxp.tile([BS, D], F32, name="x_sb", tag="wx")
    nc.sync.dma_start(x_sb[:], x_d[:])
    xp = io.tile([BS, D], F32, name="xp", tag="bxp")

    def transpose_into_zt(src, iter_tag):
        """PE-transpose src [128, 2048] into 4 zT tiles of [128, 512]."""
        zts = []
        for j in range(NCH):
            tp = tpp.tile([P, CW], F32, name=f"tp_{iter_tag}_{j}", tag="tp")
            for l in range(4):
                k = 4 * j + l
                nc.tensor.transpose(
                    tp[:, l * P:(l + 1) * P], src[:, k * P:(k + 1) * P], ident[:]
                )
            zt = ztp.tile([P, CW], F32R, name=f"zt_{iter_tag}_{j}", tag="zt")
            nc.scalar.copy(zt[:], tp[:])
            zts.append(zt)
        return zts

    # ---- transpose x for the xp GEMM ----
    xT = transpose_into_zt(x_sb, "x")

    # ---- GEMM1: xp = x @ Wx + b  (Wx streamed from HBM) ----
    mm = [mmp.tile([P, CW], F32, name=f"mm_xp_{n}", tag="mm") for n in range(NCH)]
    # bias via rank-1 matmul: ones^T (1xP) @ b (1xCW) broadcasts b to all rows
    for n in range(NCH):
        nc.tensor.matmul(
            mm[n][:], _r(ones_row[:]), _r(b_sb[:, n * CW:(n + 1) * CW]),
            start=True, stop=False,
        )
    for k in range(NK):
        wxk = wxp.tile([P, D], F32R, name=f"wx{k}", tag="wx")
        nc.sync.dma_start(wxk[:], wx_d[k * P:(k + 1) * P, :])
        j, l = k // 4, k % 4
        for n in range(NCH):
            nc.tensor.matmul(
                mm[n][:], _r(xT[j][:, l * P:(l + 1) * P]),
                _r(wxk[:, n * CW:(n + 1) * CW]),
                start=False, stop=(k == NK - 1),
            )
    for n in range(NCH):
        nc.scalar.copy(xp[:, n * CW:(n + 1) * CW], mm[n][:])

    # ---- load Wz (first NK_RES chunks resident for iterations 1..5;
    #      the rest re-streamed per iteration through the wx slots) ----
    wz = []
    for k in range(NK_RES):
        t = wzp.tile([P, D], F32R, name=f"wz{k}", tag="wz")
        nc.sync.dma_start(t[:], wz_d[k * P:(k + 1) * P, :])
        wz.append(t)

    # ---- iteration 0: z1 = 0.9 * tanh(xp) ----
    fz0 = state.tile([BS, D], F32, name="fz0", tag="g")
    nc.scalar.activation(fz0[:], xp[:], AF.Tanh, bias=zbias[:])
    z1 = state.tile([BS, D], F32, name="z1", tag="z")
    nc.vector.tensor_scalar_mul(z1[:], fz0[:], BETA)
    zT = transpose_into_zt(z1, "i0")

    # ---- iterations 1..5 ----
    z, g_prev, u_prev, sgg_prev = z1, None, None, None

    for it in range(1, MAX_ITER):
        anderson = it >= 2
        last = it == MAX_ITER - 1

        # non-resident Wz chunks re-streamed for this iteration
        wz_it = list(wz)
        for k in range(NK_RES, NK):
            t = wxp.tile([P, D], F32R, name=f"wz{k}_{it}", tag="wx")
            nc.sync.dma_start(t[:], wz_d[k * P:(k + 1) * P, :])
            wz_it.append(t)

        # GEMM: z @ Wz into 4 PSUM banks, n-outer / k-inner so bank n
        # completes early and the elementwise chain pipelines behind PE.
        mm = [mmp.tile([P, CW], F32, name=f"mm_{it}_{n}", tag="mm")
              for n in range(NCH)]
        for n in range(NCH):
            for k in range(NK):
                j, l = k // 4, k % 4
                nc.tensor.matmul(
                    mm[n][:], _r(zT[j][:, l * P:(l + 1) * P]),
                    _r(wz_it[k][:, n * CW:(n + 1) * CW]),
                    start=(k == 0), stop=(k == NK - 1),
                )

        g = state.tile([BS, D], F32, name=f"g{it}", tag="g")
        u = state.tile([BS, D], F32, name=f"u{it}", tag="u")
        sm = smp.tile([P, 24], F32, name=f"sm{it}", tag="sm")
        vs = []

        for n in range(NCH):
            sl = slice(n * CW, (n + 1) * CW)
            # pre-activation: psum += xp (in place on PSUM)
            nc.vector.tensor_add(mm[n][:], mm[n][:], xp[:, sl])
            fz_n = chk.tile([P, CW], F32, name=f"fz{it}_{n}", tag="fz")
            nc.scalar.activation(fz_n[:], mm[n][:], AF.Tanh, bias=zbias[:])
            nc.vector.tensor_sub(g[:, sl], fz_n[:], z[:, sl])
            if anderson:
                # s_ggp partials (cols 0..3): one fused product+row-sum pass
                nc.vector.scalar_tensor_tensor(
                    out=fz_n[:], in0=g[:, sl], scalar=1.0,
                    in1=g_prev[:, sl], op0=OP.mult, op1=OP.mult,
                    accum_out=sm[:, n:n + 1],
                )
            if anderson or it == 1:
                # s_gg partials via ACT square into the dead PSUM bank
                # (cols 4..7)
                nc.scalar.activation(
                    mm[n][:], g[:, sl], AF.Square, bias=zbias[:],
                    accum_out=sm[:, 4 + n:5 + n],
                )
            # u = 0.9*g + z
            nc.vector.scalar_tensor_tensor(
                out=u[:, sl], in0=g[:, sl], scalar=BETA, in1=z[:, sl],
                op0=OP.mult, op1=OP.add,
            )
            if anderson:
                v_n = chk.tile([P, CW], F32, name=f"v{it}_{n}", tag="v", bufs=4)
                nc.vector.tensor_sub(v_n[:], u[:, sl], u_prev[:, sl])
                vs.append(v_n)

        if anderson or it == 1:
            # s_gg = sum of 4 partials (cols 4..7) -> col 13
            nc.vector.tensor_add(sm[:, 11:12], sm[:, 4:5], sm[:, 5:6])
            nc.vector.tensor_add(sm[:, 12:13], sm[:, 6:7], sm[:, 7:8])
            nc.vector.tensor_add(sm[:, 13:14], sm[:, 11:12], sm[:, 12:13])

        if anderson:
            # s_ggp = sum of 4 partials (cols 0..3) -> col 10
            nc.vector.tensor_add(sm[:, 8:9], sm[:, 0:1], sm[:, 1:2])
            nc.vector.tensor_add(sm[:, 9:10], sm[:, 2:3], sm[:, 3:4])
            nc.vector.tensor_add(sm[:, 10:11], sm[:, 8:9], sm[:, 9:10])
            sggp = sm[:, 10:11]
            sgg = sm[:, 13:14]
            # num = s_gg - s_ggp            -> col 14
            nc.vector.tensor_sub(sm[:, 14:15], sgg, sggp)
            # den1 = s_gg - 2*s_ggp         -> col 15
            nc.vector.scalar_tensor_tensor(
                out=sm[:, 15:16], in0=sggp, scalar=-2.0, in1=sgg,
                op0=OP.mult, op1=OP.add,
            )
            # den = (s_gg_prev + LAM) + den1 -> col 16
            nc.vector.scalar_tensor_tensor(
                out=sm[:, 16:17], in0=sgg_prev, scalar=LAM, in1=sm[:, 15:16],
                op0=OP.add, op1=OP.add,
            )
            nc.vector.reciprocal(sm[:, 17:18], sm[:, 16:17])
            # -gamma = -num * recip          -> col 18
            nc.vector.scalar_tensor_tensor(
                out=sm[:, 18:19], in0=sm[:, 14:15], scalar=-1.0,
                in1=sm[:, 17:18], op0=OP.mult, op1=OP.mult,
            )
            ngam = sm[:, 18:19]

            z_new = state.tile([BS, D], F32, name=f"z{it + 1}", tag="z")
            for n in range(NCH):
                sl = slice(n * CW, (n + 1) * CW)
                nc.vector.scalar_tensor_tensor(
                    out=z_new[:, sl], in0=vs[n][:], scalar=ngam, in1=u[:, sl],
                    op0=OP.mult, op1=OP.add,
                )
        else:
            z_new = u  # z_{i+1} = u_i for i < 2

        if not last:
            zT = transpose_into_zt(z_new, f"i{it}")

        z, g_prev, u_prev = z_new, g, u
        if anderson or it == 1:
            sgg_prev = sm[:, 13:14]

    nc.sync.dma_start(out_d[:], z[:])


def build_kernel():
    import contextlib

    nc = bacc.Bacc("TRN2", target_bir_lowering=False, debug=False)
    x_d = nc.dram_tensor("x_s", [BS, D], F32, kind="ExternalInput").ap()
    wz_d = nc.dram_tensor("wz", [D, D], F32R, kind="ExternalInput").ap()
    wx_d = nc.dram_tensor("wx", [D, D], F32R, kind="ExternalInput").ap()
    b_d = nc.dram_tensor("b_in", [1, D], F32R, kind="ExternalInput").ap()
    out_d = nc.dram_tensor("z_out", [BS, D], F32, kind="ExternalOutput").ap()

    with tile.TileContext(nc) as tc:
        with contextlib.ExitStack() as ctx:
            _emit(tc, ctx, x_d, wz_d, wx_d, b_d, out_d)
    nc.compile()
    return nc


_built = None


def _in_maps(x, Wz, Wx, b):
    x = np.ascontiguousarray(x, dtype=np.float32)
    Wz = np.ascontiguousarray(Wz, dtype=np.float32)
    Wx = np.ascontiguousarray(Wx, dtype=np.float32)
    b = np.ascontiguousarray(b, dtype=np.float32).reshape(1, D)
    return [
        {"x_s": x[c * BS:(c + 1) * BS], "wz": Wz, "wx": Wx, "b_in": b}
        for c in range(N_CORES)
    ]


def run(x, Wz, Wx, b, trace=False):
    """Build (cached), run on 8 cores, return (output, BassKernelResults)."""
    global _built
    if _built is None:
        _built = build_kernel()
    from concourse.bass_utils import run_bass_kernel_spmd

    res = run_bass_kernel_spmd(
        _built, _in_maps(x, Wz, Wx, b), core_ids=list(range(N_CORES)),
        trace=trace,
    )
    out = np.concatenate(
        [res.results[c]["z_out"] for c in range(N_CORES)], axis=0
    )
    return out, res


def kernel(x, Wz, Wx, b):
    out, _ = run(x, Wz, Wx, b)
    return out.astype(np.float32)


# revision 22
# speedup vs baseline: 194.5577x; 194.5577x over previous
"""Trainium2 Bass kernel for the Anderson-accelerated DEQ block.

Math (refactored but numerically equivalent to the reference):
    xp = x @ Wx + b
    z_0 = 0
    for i in 0..5:
        fz = tanh(z_i @ Wz + xp)          # i==0: tanh(xp)
        g_i = fz - z_i
        u_i = z_i + 0.9 g_i
        if i < 2:  z_{i+1} = u_i
        else:
            s_gg  = rowsum(g_i * g_i)
            s_ggp = rowsum(g_i * g_{i-1})
            num   = s_gg - s_ggp                      # == rowsum(DG*g)
            den   = s_gg - 2 s_ggp + s_gg_prev + LAM  # == rowsum(DG*DG)+LAM
            gamma = num / den
            z_{i+1} = u_i - gamma * (u_i - u_{i-1})   # == z+.9g-gamma(DZ+.9DG)
    return z_6

Sharding: data-parallel over batch. 8 cores x 128 rows each; Wz/Wx/b
replicated per core. No cross-core communication.

Schedule highlights:
  - float32r matmuls (1 cycle/row, ~1e-4 precision vs fp32).
  - xp lives in PSUM across GEMMs: the bias rides a rank-1 ones x b matmul,
    iteration 1 accumulates straight on top of GEMM1's banks, and later
    iterations get xp pre-seeded into their banks by ACT copies so the
    tanh reads need no separate add pass.
  - elementwise chain is chunked (4 x 512 cols) and spread across DVE
    (g, s_ggp, z_new), ACT (tanh, square accum, zT evict, pre-seed) and
    Pool/GpSimd (u, v).
  - each z_new chunk is transposed immediately and its 16 matmuls of the
    next GEMM are emitted inline, so PE flows from transposes into the
    next GEMM without a barrier.
"""

import numpy as np

import concourse.bass as bass  # noqa: F401
import concourse.bacc as bacc
import concourse.mybir as mybir
import concourse.tile as tile
from concourse.masks import make_identity

AF = mybir.ActivationFunctionType
OP = mybir.AluOpType
F32 = mybir.dt.float32
F32R = mybir.dt.float32r

N_CORES = 8
B, D = 1024, 2048
BS = B // N_CORES       # 128 rows per core
P = 128
NK = D // P             # 16 contraction chunks
NCH = 4                 # column chunks
CW = D // NCH           # 512
NK_RES = 14             # Wz chunks in the dedicated pool (rest ride wx slots)
N_WARM = 0               # dummy PE transposes per iteration (p-state hold)
BETA = 0.9
LAM = 1e-4
MAX_ITER = 6


def _make_pools(tc, ctx):
    return dict(
        const=ctx.enter_context(tc.tile_pool(name="const", bufs=1)),
        wzp=ctx.enter_context(tc.tile_pool(name="wzp", bufs=NK_RES)),
        wxp=ctx.enter_context(tc.tile_pool(name="wxp", bufs=2)),
        state=ctx.enter_context(tc.tile_pool(name="state", bufs=2)),
        io=ctx.enter_context(tc.tile_pool(name="io", bufs=1)),
        chk=ctx.enter_context(tc.tile_pool(name="chk", bufs=2)),
        ztp=ctx.enter_context(tc.tile_pool(name="ztp", bufs=4)),
        smp=ctx.enter_context(tc.tile_pool(name="smp", bufs=2)),
        mmp=ctx.enter_context(tc.tile_pool(name="mmp", bufs=4, space="PSUM")),
        tpp=ctx.enter_context(tc.tile_pool(name="tpp", bufs=2, space="PSUM")),
        warm_ps=ctx.enter_context(tc.tile_pool(name="warm", bufs=1, space="PSUM")),
    )


def _emit(tc, pools, x_d, wz_d, wx_d, b_d, out_d):
    nc = tc.nc
    const = pools["const"]
    wzp = pools["wzp"]
    wxp = pools["wxp"]
    state = pools["state"]
    io = pools["io"]
    chk = pools["chk"]
    ztp = pools["ztp"]
    smp = pools["smp"]
    mmp = pools["mmp"]
    tpp = pools["tpp"]
    warm_ps = pools["warm_ps"]

    # constants
    ident = const.tile([P, P], F32, name="ident")
    make_identity(nc, ident)
    zbias = const.tile([P, 1], F32, name="zbias")
    nc.gpsimd.memset(zbias[:], 0.0)
    ones_f32 = const.tile([1, P], F32, name="ones_f32")
    nc.gpsimd.memset(ones_f32[:], 1.0)
    ones_row = const.tile([1, P], F32R, name="ones_row")
    nc.scalar.copy(ones_row[:], ones_f32[:])
    # b shares an io slot with xp (consumed before xp's first write)
    b_sb = io.tile([1, D], F32R, name="b_sb", tag="bxp")
    nc.sync.dma_start(b_sb[:], b_d[:])

    # x rides a Wx-stream slot: dead after the x transposes
    x_sb = wxp.tile([BS, D], F32, name="x_sb", tag="wx")
    nc.sync.dma_start(x_sb[:], x_d[:])
    xp = io.tile([BS, D], F32, name="xp", tag="bxp")

    warm = warm_ps.tile([P, P], F32, name="warm")

    def keep_warm(count, anchor):
        """Dummy PE transposes to absorb the PE p-state ramp during the
        chain stall. `anchor` is an SBUF AP produced early in the chain so
        the scheduler cannot hoist these ahead of the GEMM."""
        for i in range(count):
            nc.tensor.transpose(warm[:], anchor, ident[:])

    def transpose_group(src, n, tag):
        """Transpose src columns [n*CW,(n+1)*CW) into one zT tile."""
        tp = tpp.tile([P, CW], F32, name=f"tp_{tag}_{n}", tag="tp")
        for l in range(4):
            k = 4 * n + l
            nc.tensor.transpose(
                tp[:, l * P:(l + 1) * P], src[:, k * P:(k + 1) * P], ident[:]
            )
        zt = ztp.tile([P, CW], F32R, name=f"zt_{tag}_{n}", tag="zt")
        nc.scalar.copy(zt[:], tp[:])
        return zt

    def emit_gemm_chunk(mm, zts, n, stop):
        """16 matmuls: stationary zT group n, all 4 banks, k in 4n..4n+3."""
        for l in range(4):
            for m in range(NCH):
                nc.tensor.matmul(
                    mm[m][:], zts[n][:, l * P:(l + 1) * P],
                    wz[4 * n + l][:, m * CW:(m + 1) * CW],
                    start=False, stop=(stop and l == 3),
                    skip_group_check=True,
                )

    # ---- transpose x for the xp GEMM ----
    xT = [transpose_group(x_sb, n, "x") for n in range(NCH)]

    # ---- GEMM1: xp = x @ Wx + b  (Wx streamed from HBM) ----
    mm = [mmp.tile([P, CW], F32, name=f"mm_xp_{n}", tag="mm") for n in range(NCH)]
    # bias via rank-1 matmul: ones^T (1xP) @ b (1xCW) broadcasts b to all rows
    for n in range(NCH):
        nc.tensor.matmul(
            mm[n][:], ones_row[:], b_sb[:, n * CW:(n + 1) * CW],
            start=True, stop=False,
        )
    for k in range(NK):
        wxk = wxp.tile([P, D], F32R, name=f"wx{k}", tag="wx")
        nc.sync.dma_start(wxk[:], wx_d[k * P:(k + 1) * P, :])
        j, l = k // 4, k % 4
        for n in range(NCH):
            nc.tensor.matmul(
                mm[n][:], xT[j][:, l * P:(l + 1) * P],
                wxk[:, n * CW:(n + 1) * CW],
                start=False, stop=(k == NK - 1),
            )
    for n in range(NCH):
        nc.scalar.copy(xp[:, n * CW:(n + 1) * CW], mm[n][:])

    # ---- load Wz: NK_RES chunks in their own pool; the remaining chunks
    #      park permanently in Wx-stream slots (free after GEMM1) ----
    wz = []
    for k in range(NK_RES):
        t = wzp.tile([P, D], F32R, name=f"wz{k}", tag="wz")
        nc.sync.dma_start(t[:], wz_d[k * P:(k + 1) * P, :])
        wz.append(t)
    for k in range(NK_RES, NK):
        t = wxp.tile([P, D], F32R, name=f"wz{k}", tag="wx")
        nc.sync.dma_start(t[:], wz_d[k * P:(k + 1) * P, :])
        wz.append(t)

    # ---- iteration 0: z1 = 0.9*tanh(xp); iter-1 GEMM accumulates onto
    #      the GEMM1 banks (they already hold xp) ----
    fz0 = state.tile([BS, D], F32, name="fz0", tag="g")
    z1 = state.tile([BS, D], F32, name="z1", tag="z")
    zT = [None] * NCH
    for n in range(NCH):
        sl = slice(n * CW, (n + 1) * CW)
        nc.scalar.activation(fz0[:, sl], xp[:, sl], AF.Tanh, bias=zbias[:])
        nc.vector.tensor_scalar_mul(z1[:, sl], fz0[:, sl], BETA)
        zT[n] = transpose_group(z1, n, "i0")
    for n in range(NCH):
        emit_gemm_chunk(mm, zT, n, stop=(n == NCH - 1))

    # ---- iterations 1..5 ----
    z, g_prev, u_prev, sgg_prev = z1, None, None, None

    for it in range(1, MAX_ITER):
        anderson = it >= 2
        last = it == MAX_ITER - 1

        g = state.tile([BS, D], F32, name=f"g{it}", tag="g")
        u = state.tile([BS, D], F32, name=f"u{it}", tag="u")
        sm = smp.tile([P, 24], F32, name=f"sm{it}", tag="sm")
        vs = []
        if not last:
            mm_next = [mmp.tile([P, CW], F32, name=f"mm_{it + 1}_{n}", tag="mm")
                       for n in range(NCH)]

        for n in range(NCH):
            sl = slice(n * CW, (n + 1) * CW)
            fz_n = chk.tile([P, CW], F32, name=f"fz{it}_{n}", tag="fz")
            nc.scalar.activation(fz_n[:], mm[n][:], AF.Tanh, bias=zbias[:])
            if n == 0 and not last:
                keep_warm(N_WARM, fz_n[:, 0:P])
            if not last:
                # bank n is dead: pre-seed xp for the next GEMM
                nc.scalar.copy(mm_next[n][:], xp[:, sl])
            nc.vector.tensor_sub(g[:, sl], fz_n[:], z[:, sl])
            if anderson:
                # s_ggp partials (cols 0..3): fused product + row-sum
                nc.vector.scalar_tensor_tensor(
                    out=fz_n[:], in0=g[:, sl], scalar=1.0,
                    in1=g_prev[:, sl], op0=OP.mult, op1=OP.mult,
                    accum_out=sm[:, n:n + 1],
                )
            if anderson or it == 1:
                # s_gg partials (cols 4..7) on ACT
                dmp = chk.tile([P, CW], F32, name=f"dmp{it}_{n}", tag="dmp",
                               bufs=1)
                nc.scalar.activation(
                    dmp[:], g[:, sl], AF.Square, bias=zbias[:],
                    accum_out=sm[:, 4 + n:5 + n],
                )
            # u = 0.9*g + z
            nc.vector.scalar_tensor_tensor(
                out=u[:, sl], in0=g[:, sl], scalar=BETA, in1=z[:, sl],
                op0=OP.mult, op1=OP.add,
            )
            if anderson:
                v_n = chk.tile([P, CW], F32, name=f"v{it}_{n}", tag="v", bufs=4)
                nc.vector.tensor_sub(v_n[:], u[:, sl], u_prev[:, sl])
                vs.append(v_n)

        if anderson or it == 1:
            # s_gg = sum of 4 partials (cols 4..7) -> col 13
            nc.vector.tensor_add(sm[:, 11:12], sm[:, 4:5], sm[:, 5:6])
            nc.vector.tensor_add(sm[:, 12:13], sm[:, 6:7], sm[:, 7:8])
            nc.vector.tensor_add(sm[:, 13:14], sm[:, 11:12], sm[:, 12:13])

        if anderson:
            # s_ggp = sum of 4 partials (cols 0..3) -> col 10
            nc.vector.tensor_add(sm[:, 8:9], sm[:, 0:1], sm[:, 1:2])
            nc.vector.tensor_add(sm[:, 9:10], sm[:, 2:3], sm[:, 3:4])
            nc.vector.tensor_add(sm[:, 10:11], sm[:, 8:9], sm[:, 9:10])
            sggp = sm[:, 10:11]
            sgg = sm[:, 13:14]
            nc.vector.tensor_sub(sm[:, 14:15], sgg, sggp)            # num
            nc.vector.scalar_tensor_tensor(                          # den1
                out=sm[:, 15:16], in0=sggp, scalar=-2.0, in1=sgg,
                op0=OP.mult, op1=OP.add,
            )
            nc.vector.scalar_tensor_tensor(                          # den
                out=sm[:, 16:17], in0=sgg_prev, scalar=LAM, in1=sm[:, 15:16],
                op0=OP.add, op1=OP.add,
            )
            nc.vector.reciprocal(sm[:, 17:18], sm[:, 16:17])
            nc.vector.scalar_tensor_tensor(                          # -gamma
                out=sm[:, 18:19], in0=sm[:, 14:15], scalar=-1.0,
                in1=sm[:, 17:18], op0=OP.mult, op1=OP.mult,
            )
            ngam = sm[:, 18:19]
            z_new = state.tile([BS, D], F32, name=f"z{it + 1}", tag="z")
        else:
            z_new = u  # z_{i+1} = u_i for i < 2

        zT = [None] * NCH
        for n in range(NCH):
            sl = slice(n * CW, (n + 1) * CW)
            if anderson:
                nc.vector.scalar_tensor_tensor(
                    out=z_new[:, sl], in0=vs[n][:], scalar=ngam, in1=u[:, sl],
                    op0=OP.mult, op1=OP.add,
                )
            if last:
                nc.sync.dma_start(out_d[:, sl], z_new[:, sl])
            else:
                zT[n] = transpose_group(z_new, n, f"i{it}")
        if not last:
            for n in range(NCH):
                emit_gemm_chunk(mm_next, zT, n, stop=(n == NCH - 1))

        if not last:
            mm = mm_next
        z, g_prev, u_prev = z_new, g, u
        if anderson or it == 1:
            sgg_prev = sm[:, 13:14]


def build_kernel(repeat=1):
    import contextlib

    nc = bacc.Bacc("TRN2", target_bir_lowering=False, debug=False)
    x_d = nc.dram_tensor("x_s", [BS, D], F32, kind="ExternalInput").ap()
    wz_d = nc.dram_tensor("wz", [D, D], F32R, kind="ExternalInput").ap()
    wx_d = nc.dram_tensor("wx", [D, D], F32R, kind="ExternalInput").ap()
    b_d = nc.dram_tensor("b_in", [1, D], F32R, kind="ExternalInput").ap()
    out_d = nc.dram_tensor("z_out", [BS, D], F32, kind="ExternalOutput").ap()

    with tile.TileContext(nc) as tc:
        with contextlib.ExitStack() as ctx:
            pools = _make_pools(tc, ctx)
            if repeat == 1:
                _emit(tc, pools, x_d, wz_d, wx_d, b_d, out_d)
            else:
                with tc.For_i(0, repeat, 1):
                    _emit(tc, pools, x_d, wz_d, wx_d, b_d, out_d)
    nc.compile()
    return nc


_built = None


def _in_maps(x, Wz, Wx, b):
    x = np.ascontiguousarray(x, dtype=np.float32)
    Wz = np.ascontiguousarray(Wz, dtype=np.float32)
    Wx = np.ascontiguousarray(Wx, dtype=np.float32)
    b = np.ascontiguousarray(b, dtype=np.float32).reshape(1, D)
    return [
        {"x_s": x[c * BS:(c + 1) * BS], "wz": Wz, "wx": Wx, "b_in": b}
        for c in range(N_CORES)
    ]


def run(x, Wz, Wx, b, trace=False):
    """Build (cached), run on 8 cores, return (output, BassKernelResults)."""
    global _built
    if _built is None:
        _built = build_kernel()
    from concourse.bass_utils import run_bass_kernel_spmd

    res = run_bass_kernel_spmd(
        _built, _in_maps(x, Wz, Wx, b), core_ids=list(range(N_CORES)),
        trace=trace,
    )
    out = np.concatenate(
        [res.results[c]["z_out"] for c in range(N_CORES)], axis=0
    )
    return out, res


def kernel(x, Wz, Wx, b):
    out, _ = run(x, Wz, Wx, b)
    return out.astype(np.float32)


# revision 24
# speedup vs baseline: 388.2167x; 1.9954x over previous
"""Trainium2 Bass kernel for the Anderson-accelerated DEQ block.

Math (refactored but numerically equivalent to the reference):
    xp = x @ Wx + b
    z_0 = 0
    for i in 0..5:
        fz = tanh(z_i @ Wz + xp)          # i==0: tanh(xp)
        g_i = fz - z_i
        u_i = z_i + 0.9 g_i
        if i < 2:  z_{i+1} = u_i
        else:
            s_gg  = rowsum(g_i * g_i)
            s_ggp = rowsum(g_i * g_{i-1})
            num   = s_gg - s_ggp                      # == rowsum(DG*g)
            den   = s_gg - 2 s_ggp + s_gg_prev + LAM  # == rowsum(DG*DG)+LAM
            gamma = num / den
            z_{i+1} = u_i - gamma * (u_i - u_{i-1})   # == z+.9g-gamma(DZ+.9DG)
    return z_6

Sharding: data-parallel over batch. 8 cores x 128 rows each; Wz/Wx/b
replicated per core. No cross-core communication.

Schedule highlights:
  - float32r matmuls (1 cycle/row, ~1e-4 precision vs fp32).
  - xp lives in PSUM across GEMMs: the bias rides a rank-1 ones x b matmul,
    iteration 1 accumulates straight on top of GEMM1's banks, and later
    iterations get xp pre-seeded into their banks by ACT copies so the
    tanh reads need no separate add pass.
  - elementwise chain is chunked (4 x 512 cols) and split across DVE
    (g, s_ggp fused product+row-sum, u, v, z_new, gamma scalars) and ACT
    (tanh, square accum, zT evict, xp pre-seed), pipelining behind the
    n-outer/k-inner GEMM whose banks complete progressively.
  - z is transposed for the next GEMM's stationary operand by PE
    (16 transposes + 4 ACT evictions that also round f32 -> f32r).
"""

import numpy as np

import concourse.bass as bass  # noqa: F401
import concourse.bacc as bacc
import concourse.mybir as mybir
import concourse.tile as tile
from concourse.masks import make_identity

AF = mybir.ActivationFunctionType
OP = mybir.AluOpType
F32 = mybir.dt.float32
F32R = mybir.dt.float32r

N_CORES = 8
B, D = 1024, 2048
BS = B // N_CORES       # 128 rows per core
P = 128
NK = D // P             # 16 contraction chunks
NCH = 4                 # column chunks
CW = D // NCH           # 512
NK_RES = 14             # Wz chunks in the dedicated pool (rest ride wx slots)
N_WARM = 16              # dummy PE transposes per iteration (p-state hold)
BETA = 0.9
LAM = 1e-4
MAX_ITER = 6


def _make_pools(tc, ctx):
    return dict(
        const=ctx.enter_context(tc.tile_pool(name="const", bufs=1)),
        wzp=ctx.enter_context(tc.tile_pool(name="wzp", bufs=NK_RES)),
        wxp=ctx.enter_context(tc.tile_pool(name="wxp", bufs=2)),
        state=ctx.enter_context(tc.tile_pool(name="state", bufs=2)),
        io=ctx.enter_context(tc.tile_pool(name="io", bufs=1)),
        chk=ctx.enter_context(tc.tile_pool(name="chk", bufs=2)),
        ztp=ctx.enter_context(tc.tile_pool(name="ztp", bufs=4)),
        smp=ctx.enter_context(tc.tile_pool(name="smp", bufs=2)),
        mmp=ctx.enter_context(tc.tile_pool(name="mmp", bufs=4, space="PSUM")),
        tpp=ctx.enter_context(tc.tile_pool(name="tpp", bufs=2, space="PSUM")),
        warm_ps=ctx.enter_context(tc.tile_pool(name="warm", bufs=1, space="PSUM")),
    )


def _emit(tc, pools, x_d, wz_d, wx_d, b_d, out_d):
    nc = tc.nc
    const = pools["const"]
    wzp = pools["wzp"]
    wxp = pools["wxp"]
    state = pools["state"]
    io = pools["io"]
    chk = pools["chk"]
    ztp = pools["ztp"]
    smp = pools["smp"]
    mmp = pools["mmp"]
    tpp = pools["tpp"]
    warm_ps = pools["warm_ps"]

    # constants
    ident = const.tile([P, P], F32, name="ident")
    make_identity(nc, ident)
    zbias = const.tile([P, 1], F32, name="zbias")
    nc.gpsimd.memset(zbias[:], 0.0)
    ones_f32 = const.tile([1, P], F32, name="ones_f32")
    nc.gpsimd.memset(ones_f32[:], 1.0)
    ones_row = const.tile([1, P], F32R, name="ones_row")
    nc.scalar.copy(ones_row[:], ones_f32[:])
    # b shares an io slot with xp (consumed before xp's first write)
    b_sb = io.tile([1, D], F32R, name="b_sb", tag="bxp")
    nc.sync.dma_start(b_sb[:], b_d[:])

    # x rides a Wx-stream slot: dead after the x transposes
    x_sb = wxp.tile([BS, D], F32, name="x_sb", tag="wx")
    nc.sync.dma_start(x_sb[:], x_d[:])
    xp = io.tile([BS, D], F32, name="xp", tag="bxp")

    warm = warm_ps.tile([P, P], F32, name="warm")

    def keep_warm(count, anchor):
        """Dummy PE transposes to absorb the PE p-state ramp during the
        chain stall. `anchor` is an SBUF AP produced early in the chain so
        the scheduler cannot hoist these ahead of the GEMM."""
        for i in range(count):
            nc.tensor.transpose(warm[:], anchor, ident[:])

    def transpose_group(src, n, tag):
        """Transpose src columns [n*CW,(n+1)*CW) into one zT tile."""
        tp = tpp.tile([P, CW], F32, name=f"tp_{tag}_{n}", tag="tp")
        for l in range(4):
            k = 4 * n + l
            nc.tensor.transpose(
                tp[:, l * P:(l + 1) * P], src[:, k * P:(k + 1) * P], ident[:]
            )
        zt = ztp.tile([P, CW], F32R, name=f"zt_{tag}_{n}", tag="zt")
        nc.scalar.copy(zt[:], tp[:])
        return zt

    def emit_gemm_chunk(mm, zts, n, stop):
        """16 matmuls: stationary zT group n, all 4 banks, k in 4n..4n+3."""
        for l in range(4):
            for m in range(NCH):
                nc.tensor.matmul(
                    mm[m][:], zts[n][:, l * P:(l + 1) * P],
                    wz[4 * n + l][:, m * CW:(m + 1) * CW],
                    start=False, stop=(stop and l == 3),
                    skip_group_check=True,
                )

    # ---- transpose x for the xp GEMM ----
    xT = [transpose_group(x_sb, n, "x") for n in range(NCH)]

    # ---- GEMM1: xp = x @ Wx + b  (Wx streamed from HBM) ----
    mm = [mmp.tile([P, CW], F32, name=f"mm_xp_{n}", tag="mm") for n in range(NCH)]
    # bias via rank-1 matmul: ones^T (1xP) @ b (1xCW) broadcasts b to all rows
    for n in range(NCH):
        nc.tensor.matmul(
            mm[n][:], ones_row[:], b_sb[:, n * CW:(n + 1) * CW],
            start=True, stop=False,
        )
    for k in range(NK):
        wxk = wxp.tile([P, D], F32R, name=f"wx{k}", tag="wx")
        nc.sync.dma_start(wxk[:], wx_d[k * P:(k + 1) * P, :])
        j, l = k // 4, k % 4
        for n in range(NCH):
            nc.tensor.matmul(
                mm[n][:], xT[j][:, l * P:(l + 1) * P],
                wxk[:, n * CW:(n + 1) * CW],
                start=False, stop=(k == NK - 1),
            )
    for n in range(NCH):
        nc.scalar.copy(xp[:, n * CW:(n + 1) * CW], mm[n][:])

    # ---- load Wz: NK_RES chunks in their own pool; the remaining chunks
    #      park permanently in Wx-stream slots (free after GEMM1) ----
    wz = []
    for k in range(NK_RES):
        t = wzp.tile([P, D], F32R, name=f"wz{k}", tag="wz")
        nc.sync.dma_start(t[:], wz_d[k * P:(k + 1) * P, :])
        wz.append(t)
    for k in range(NK_RES, NK):
        t = wxp.tile([P, D], F32R, name=f"wz{k}", tag="wx")
        nc.sync.dma_start(t[:], wz_d[k * P:(k + 1) * P, :])
        wz.append(t)

    # ---- iteration 0: z1 = 0.9*tanh(xp); iter-1 GEMM accumulates onto
    #      the GEMM1 banks (they already hold xp) ----
    fz0 = state.tile([BS, D], F32, name="fz0", tag="g")
    z1 = state.tile([BS, D], F32, name="z1", tag="z")
    zT = [None] * NCH
    for n in range(NCH):
        sl = slice(n * CW, (n + 1) * CW)
        nc.scalar.activation(fz0[:, sl], xp[:, sl], AF.Tanh, bias=zbias[:])
        nc.vector.tensor_scalar_mul(z1[:, sl], fz0[:, sl], BETA)
        zT[n] = transpose_group(z1, n, "i0")
    for n in range(NCH):
        emit_gemm_chunk(mm, zT, n, stop=(n == NCH - 1))

    # ---- iterations 1..5 ----
    z, g_prev, u_prev, sgg_prev = z1, None, None, None

    for it in range(1, MAX_ITER):
        anderson = it >= 2
        last = it == MAX_ITER - 1

        g = state.tile([BS, D], F32, name=f"g{it}", tag="g")
        u = state.tile([BS, D], F32, name=f"u{it}", tag="u")
        sm = smp.tile([P, 24], F32, name=f"sm{it}", tag="sm")
        vs = []
        if not last:
            mm_next = [mmp.tile([P, CW], F32, name=f"mm_{it + 1}_{n}", tag="mm")
                       for n in range(NCH)]

        for n in range(NCH):
            sl = slice(n * CW, (n + 1) * CW)
            fz_n = chk.tile([P, CW], F32, name=f"fz{it}_{n}", tag="fz")
            nc.scalar.activation(fz_n[:], mm[n][:], AF.Tanh, bias=zbias[:])
            if n == 0 and not last:
                keep_warm(N_WARM, fz_n[:, 0:P])
            if not last:
                # bank n is dead: pre-seed xp for the next GEMM
                nc.scalar.copy(mm_next[n][:], xp[:, sl])
            nc.vector.tensor_sub(g[:, sl], fz_n[:], z[:, sl])
            if anderson:
                # s_ggp partials (cols 0..3): fused product + row-sum
                nc.vector.scalar_tensor_tensor(
                    out=fz_n[:], in0=g[:, sl], scalar=1.0,
                    in1=g_prev[:, sl], op0=OP.mult, op1=OP.mult,
                    accum_out=sm[:, n:n + 1],
                )
            if anderson or it == 1:
                # s_gg partials (cols 4..7) on ACT
                dmp = chk.tile([P, CW], F32, name=f"dmp{it}_{n}", tag="dmp",
                               bufs=1)
                nc.scalar.activation(
                    dmp[:], g[:, sl], AF.Square, bias=zbias[:],
                    accum_out=sm[:, 4 + n:5 + n],
                )
            # u = 0.9*g + z
            nc.vector.scalar_tensor_tensor(
                out=u[:, sl], in0=g[:, sl], scalar=BETA, in1=z[:, sl],
                op0=OP.mult, op1=OP.add,
            )
            if anderson:
                v_n = chk.tile([P, CW], F32, name=f"v{it}_{n}", tag="v", bufs=4)
                nc.vector.tensor_sub(v_n[:], u[:, sl], u_prev[:, sl])
                vs.append(v_n)

        if anderson or it == 1:
            # s_gg = sum of 4 partials (cols 4..7) -> col 13
            nc.vector.tensor_add(sm[:, 11:12], sm[:, 4:5], sm[:, 5:6])
            nc.vector.tensor_add(sm[:, 12:13], sm[:, 6:7], sm[:, 7:8])
            nc.vector.tensor_add(sm[:, 13:14], sm[:, 11:12], sm[:, 12:13])

        if anderson:
            # s_ggp = sum of 4 partials (cols 0..3) -> col 10
            nc.vector.tensor_add(sm[:, 8:9], sm[:, 0:1], sm[:, 1:2])
            nc.vector.tensor_add(sm[:, 9:10], sm[:, 2:3], sm[:, 3:4])
            nc.vector.tensor_add(sm[:, 10:11], sm[:, 8:9], sm[:, 9:10])
            sggp = sm[:, 10:11]
            sgg = sm[:, 13:14]
            nc.vector.tensor_sub(sm[:, 14:15], sgg, sggp)            # num
            nc.vector.scalar_tensor_tensor(                          # den1
                out=sm[:, 15:16], in0=sggp, scalar=-2.0, in1=sgg,
                op0=OP.mult, op1=OP.add,
            )
            nc.vector.scalar_tensor_tensor(                          # den
                out=sm[:, 16:17], in0=sgg_prev, scalar=LAM, in1=sm[:, 15:16],
                op0=OP.add, op1=OP.add,
            )
            nc.vector.reciprocal(sm[:, 17:18], sm[:, 16:17])
            nc.vector.scalar_tensor_tensor(                          # -gamma
                out=sm[:, 18:19], in0=sm[:, 14:15], scalar=-1.0,
                in1=sm[:, 17:18], op0=OP.mult, op1=OP.mult,
            )
            ngam = sm[:, 18:19]
            z_new = state.tile([BS, D], F32, name=f"z{it + 1}", tag="z")
        else:
            z_new = u  # z_{i+1} = u_i for i < 2

        zT = [None] * NCH
        for n in range(NCH):
            sl = slice(n * CW, (n + 1) * CW)
            if anderson:
                nc.vector.scalar_tensor_tensor(
                    out=z_new[:, sl], in0=vs[n][:], scalar=ngam, in1=u[:, sl],
                    op0=OP.mult, op1=OP.add,
                )
            if last:
                nc.sync.dma_start(out_d[:, sl], z_new[:, sl])
            else:
                zT[n] = transpose_group(z_new, n, f"i{it}")
        if not last:
            for n in range(NCH):
                emit_gemm_chunk(mm_next, zT, n, stop=(n == NCH - 1))

        if not last:
            mm = mm_next
        z, g_prev, u_prev = z_new, g, u
        if anderson or it == 1:
            sgg_prev = sm[:, 13:14]


def build_kernel(repeat=1):
    import contextlib

    nc = bacc.Bacc("TRN2", target_bir_lowering=False, debug=False)
    x_d = nc.dram_tensor("x_s", [BS, D], F32, kind="ExternalInput").ap()
    wz_d = nc.dram_tensor("wz", [D, D], F32R, kind="ExternalInput").ap()
    wx_d = nc.dram_tensor("wx", [D, D], F32R, kind="ExternalInput").ap()
    b_d = nc.dram_tensor("b_in", [1, D], F32R, kind="ExternalInput").ap()
    out_d = nc.dram_tensor("z_out", [BS, D], F32, kind="ExternalOutput").ap()

    with tile.TileContext(nc) as tc:
        with contextlib.ExitStack() as ctx:
            pools = _make_pools(tc, ctx)
            if repeat == 1:
                _emit(tc, pools, x_d, wz_d, wx_d, b_d, out_d)
            else:
                with tc.For_i(0, repeat, 1):
                    _emit(tc, pools, x_d, wz_d, wx_d, b_d, out_d)
    nc.compile()
    return nc


_built = None


def _in_maps(x, Wz, Wx, b):
    x = np.ascontiguousarray(x, dtype=np.float32)
    Wz = np.ascontiguousarray(Wz, dtype=np.float32)
    Wx = np.ascontiguousarray(Wx, dtype=np.float32)
    b = np.ascontiguousarray(b, dtype=np.float32).reshape(1, D)
    return [
        {"x_s": x[c * BS:(c + 1) * BS], "wz": Wz, "wx": Wx, "b_in": b}
        for c in range(N_CORES)
    ]


def run(x, Wz, Wx, b, trace=False):
    """Build (cached), run on 8 cores, return (output, BassKernelResults)."""
    global _built
    if _built is None:
        _built = build_kernel()
    from concourse.bass_utils import run_bass_kernel_spmd

    res = run_bass_kernel_spmd(
        _built, _in_maps(x, Wz, Wx, b), core_ids=list(range(N_CORES)),
        trace=trace,
    )
    out = np.concatenate(
        [res.results[c]["z_out"] for c in range(N_CORES)], axis=0
    )
    return out, res


def kernel(x, Wz, Wx, b):
    out, _ = run(x, Wz, Wx, b)
    return out.astype(np.float32)
